# revision 11
# baseline (speedup 1.0000x reference)
"""Exaone4 attention kernel for 8 Trainium2 NeuronCores.

Sharding: tensor-parallel over heads (TP=8). Core i owns query heads
4i..4i+3 and kv head i (one GQA group), processes both batch elements,
and computes a row-parallel partial of the output projection; the host
sums the 8 partials.

v3: bf16 matmul pipeline (fp32 PSUM accumulation); hidden-state chunk
resident in SBUF; QKV epilogue software-pipelined behind the next
group's matmuls; attention score/sum/PV matmuls trimmed to the exact
sliding-window band (per-element PSUM accumulate); per-head softmax
denominators in 32-strided rows of one PSUM bank, inverted once per
chunk with reciprocal_approx_fast; SBUF-resident attention output;
bf16 partial-output write.

Shapes (hardcoded): B=2, S=2048, H=4096, NH=32, NKV=8, D=128,
WINDOW=1024, eps=1e-5, theta=10000.
"""

import os
import sys

for _p in ("/opt/trn_rl_repo",):
    if _p not in sys.path and os.path.isdir(_p):
        sys.path.insert(0, _p)

import numpy as np

B, S, H = 2, 2048, 4096
NH, NKV, D = 32, 8, 128
WINDOW = 1024
EPS = 1e-5
THETA = 10000.0

NCORES = 8
HPC = NH // NCORES          # query heads per core = 4
QW = HPC * D                # q-proj cols per core = 512
CH = 512                    # sequence chunk
NSC = S // CH               # 4 chunks
HC = H // 128               # 32 contraction chunks
NEG = -1.0e30

_CACHE = {}


def _build():
    import concourse.bass as bass
    import concourse.tile as tile
    from concourse import mybir, bacc

    F32 = mybir.dt.float32
    F32R = mybir.dt.float32r
    BF16 = mybir.dt.bfloat16
    EXP = mybir.ActivationFunctionType.Exp
    RSQRT = mybir.ActivationFunctionType.Abs_reciprocal_sqrt

    nc = bacc.Bacc("TRN2", target_bir_lowering=False, debug=False)

    hsT = nc.dram_tensor("hsT", [B, H, S], BF16, kind="ExternalInput")
    wq_s = nc.dram_tensor("wq_s", [H, QW], BF16, kind="ExternalInput")
    wk_s = nc.dram_tensor("wk_s", [H, D], BF16, kind="ExternalInput")
    wv_s = nc.dram_tensor("wv_s", [H, D], BF16, kind="ExternalInput")
    wo_s = nc.dram_tensor("wo_s", [QW, H], BF16, kind="ExternalInput")
    cosT = nc.dram_tensor("cosT", [D, S], BF16, kind="ExternalInput")
    sinT = nc.dram_tensor("sinT", [D, S], BF16, kind="ExternalInput")
    wrow_q = nc.dram_tensor("wrow_q", [1, D], F32R, kind="ExternalInput")
    wrow_k = nc.dram_tensor("wrow_k", [1, D], F32R, kind="ExternalInput")
    ones128 = nc.dram_tensor("ones128", [128, 1], BF16, kind="ExternalInput")
    ones_col = nc.dram_tensor("ones_col", [1, 128], F32R, kind="ExternalInput")
    protT = nc.dram_tensor("protT", [128, 128], BF16, kind="ExternalInput")
    idn = nc.dram_tensor("idn", [128, 128], BF16, kind="ExternalInput")
    mask_c = nc.dram_tensor("mask_c", [128, 128], F32, kind="ExternalInput")
    mask_w = nc.dram_tensor("mask_w", [128, 128], F32, kind="ExternalInput")
    out_part = nc.dram_tensor("out_part", [B, S, H], BF16,
                              kind="ExternalOutput")

    DEBUG = bool(os.environ.get("BASS_KERNEL_DEBUG"))
    if DEBUG:
        dbg_k = nc.dram_tensor("dbg_k", [128, S], BF16, kind="ExternalOutput")
        dbg_v = nc.dram_tensor("dbg_v", [128, S // 128, 128], BF16,
                               kind="ExternalOutput")
        dbg_q = nc.dram_tensor("dbg_q", [HPC, 128, S], BF16,
                               kind="ExternalOutput")
        dbg_a = nc.dram_tensor("dbg_a", [HPC, 128, S], BF16,
                               kind="ExternalOutput")

    with tile.TileContext(nc) as tc, \
         nc.allow_low_precision(reason="deliberate bf16 matmul pipeline"):
        with tc.tile_pool(name="consts", bufs=1) as consts, \
             tc.tile_pool(name="afin", bufs=1) as afp:
            cos_sb = consts.tile([D, S], BF16)
            sin_sb = consts.tile([D, S], BF16)
            wq_row = consts.tile([1, D], F32R)
            wk_row = consts.tile([1, D], F32R)
            on128 = consts.tile([128, 1], BF16)
            oncol = consts.tile([1, 128], F32R)
            prot = consts.tile([128, 128], BF16)
            iden = consts.tile([128, 128], BF16)
            mc = consts.tile([128, 128], F32)
            mw = consts.tile([128, 128], F32)
            eps_t = consts.tile([1, 1], F32)
            nc.vector.memset(eps_t, EPS)

            def load_consts():
                nc.sync.dma_start(cos_sb, cosT.ap())
                nc.sync.dma_start(sin_sb, sinT.ap())
                nc.sync.dma_start(wq_row, wrow_q.ap())
                nc.sync.dma_start(wk_row, wrow_k.ap())
                nc.sync.dma_start(on128, ones128.ap())
                nc.sync.dma_start(oncol, ones_col.ap())
                nc.sync.dma_start(prot, protT.ap())
                nc.sync.dma_start(iden, idn.ap())
                nc.sync.dma_start(mc, mask_c.ap())
                nc.sync.dma_start(mw, mask_w.ap())

            # attention output, SBUF-resident across phases A+B and C
            afin = [afp.tile([128, HPC, S], BF16, tag=f"af{b}",
                             name=f"af{b}") for b in range(B)]

            # ---------------- phases A+B: QKV + norm/rope + attention ----
            with tc.tile_pool(name="wqkv", bufs=1) as wp, \
                 tc.tile_pool(name="kv", bufs=1) as kvp, \
                 tc.tile_pool(name="work", bufs=2) as wrk, \
                 tc.tile_pool(name="epi", bufs=1) as epi, \
                 tc.tile_pool(name="hs", bufs=2) as hsp, \
                 tc.tile_pool(name="probs", bufs=3) as prp, \
                 tc.tile_pool(name="qf", bufs=2) as qfp, \
                 tc.tile_pool(name="aun", bufs=1) as aup, \
                 tc.tile_pool(name="ps_qkv", bufs=1, space="PSUM") as pq, \
                 tc.tile_pool(name="ps_s", bufs=3, space="PSUM") as pss, \
                 tc.tile_pool(name="ps_o", bufs=1, space="PSUM") as pso, \
                 tc.tile_pool(name="ps_sum", bufs=1, space="PSUM") as psum_p:
                paux = pss

                wq_sb = wp.tile([128, HC, QW], BF16)
                wk_sb = wp.tile([128, HC, D], BF16)
                wv_sb = wp.tile([128, HC, D], BF16)

                def load_weights(phase):
                    if phase == 0:
                        nc.sync.dma_start(
                            wq_sb[:, :, 0:128],
                            wq_s.ap()[:, 0:128]
                                .rearrange("(o p) c -> p o c", p=128))
                        nc.sync.dma_start(
                            wk_sb,
                            wk_s.ap().rearrange("(o p) c -> p o c", p=128))
                        nc.sync.dma_start(
                            wv_sb,
                            wv_s.ap().rearrange("(o p) c -> p o c", p=128))
                    else:
                        for _blk in range(1, HPC):
                            nc.sync.dma_start(
                                wq_sb[:, :, 128 * _blk:128 * (_blk + 1)],
                                wq_s.ap()[:, 128 * _blk:128 * (_blk + 1)]
                                    .rearrange("(o p) c -> p o c", p=128))

                chunks = [(b, sc) for b in range(B) for sc in range(NSC)]
                NCH = len(chunks)
                hs_state, raw_state, qf_state = {}, {}, {}
                kv_state = {}
                deferred = []

                def load_hs(ci):
                    b, sc = chunks[ci]
                    t = hsp.tile([128, HC, CH], BF16, tag="hst")
                    nc.sync.dma_start(
                        t, hsT.ap()[b, :, CH * sc:CH * (sc + 1)]
                            .rearrange("(o p) c -> p o c", p=128))
                    hs_state[ci] = t

                def emit_qkv(ci, grp, mid=None):
                    """QKV matmuls for 3 col-blocks + PSUM drain copies."""
                    b, sc = chunks[ci]
                    if grp == 0 and ci + 1 < NCH:
                        load_hs(ci + 1)
                    hs_sb = hs_state[ci] if grp == 0 else hs_state.pop(ci)
                    with nc.named_scope("qkv"):
                        qkv_ps = pq.tile([128, 3, CH], F32, tag="qkv",
                                         name=f"qkv{ci}_{grp}")
                        for hc in range(HC):
                            if hc == 4:
                                while deferred:
                                    deferred.pop(0)()
                            if mid is not None and hc in mid:
                                mid[hc]()
                            for bi in range(3):
                                blk = 3 * grp + bi
                                if blk < HPC:
                                    lhs = wq_sb[:, hc,
                                                128 * blk:128 * (blk + 1)]
                                elif blk == HPC:
                                    lhs = wk_sb[:, hc, :]
                                else:
                                    lhs = wv_sb[:, hc, :]
                                nc.tensor.matmul(
                                    qkv_ps[:, bi, :], lhs, hs_sb[:, hc, :],
                                    start=(hc == 0), stop=(hc == HC - 1))
                        raws = {}
                        for bi in range(3):
                            raw = epi.tile([128, CH], BF16,
                                           tag=f"raw{grp}{bi}",
                                           name=f"raw{grp}{bi}")
                            nc.vector.tensor_copy(raw, qkv_ps[:, bi, :])
                            raws[bi] = raw
                        raw_state[(ci, grp)] = raws

                def epi_stages(ci, grp):
                    """norm + rope for one group's blocks, split into 3
                    stages so the PE ops spread through the host matmul
                    stream (ACT rsqrt latency hides between stages)."""
                    b, sc = chunks[ci]
                    s0 = CH * sc
                    st = {}

                    def stage1():
                        raws = raw_state.pop((ci, grp))
                        st["raws"] = raws
                        if grp == 1 and sc == 0:
                            k_full = kvp.tile([128, S], BF16, tag="k_full",
                                              name=f"k{b}")
                            v_full = kvp.tile([128, S // 128, 128], BF16,
                                              tag="v_full", name=f"v{b}")
                            kv_state[b] = (k_full, v_full)
                        norm_bis = list(range(3)) if grp == 0 else [0, 1]
                        st["norm_bis"] = norm_bis
                        sqs, ssqs, rinvs = {}, {}, {}
                        for bi in norm_bis:
                            sq = epi.tile([128, CH], BF16, tag=f"sq{bi}",
                                          name=f"sq{bi}")
                            nc.vector.tensor_mul(sq, raws[bi], raws[bi])
                            sqs[bi] = sq
                        for bi in norm_bis:
                            ssq = paux.tile([1, CH], F32, tag="s",
                                            name=f"ssq{bi}")
                            nc.tensor.matmul(ssq, on128, sqs[bi],
                                             start=True, stop=True)
                            ssqs[bi] = ssq
                        for bi in norm_bis:
                            rinv = epi.tile([1, CH], F32R, tag=f"rinv{bi}",
                                            name=f"rinv{bi}")
                            nc.scalar.activation(rinv, ssqs[bi], RSQRT,
                                                 bias=eps_t, scale=1.0 / D)
                            rinvs[bi] = rinv
                        st["rinvs"] = rinvs

                    def stage2():
                        raws, rinvs = st["raws"], st["rinvs"]
                        wtils, qhats = {}, {}
                        for bi in st["norm_bis"]:
                            wrow = (wq_row if 3 * grp + bi < HPC
                                    else wk_row)
                            wtil = paux.tile([128, CH], F32, tag="s",
                                             name=f"wtil{bi}")
                            nc.tensor.matmul(wtil, wrow, rinvs[bi],
                                             start=True, stop=True)
                            wtils[bi] = wtil
                        for bi in st["norm_bis"]:
                            qhat = epi.tile([128, CH], BF16,
                                            tag=f"qhat{bi}",
                                            name=f"qhat{bi}")
                            nc.vector.tensor_mul(qhat, wtils[bi], raws[bi])
                            qhats[bi] = qhat
                        st["qhats"] = qhats

                    def stage3():
                        raws, qhats = st["raws"], st["qhats"]
                        rots = {}
                        for bi in st["norm_bis"]:
                            rot = paux.tile([128, CH], F32, tag="s",
                                            name=f"rot{bi}")
                            nc.tensor.matmul(rot, prot, qhats[bi],
                                             start=True, stop=True)
                            rots[bi] = rot
                        for bi in st["norm_bis"]:
                            blk = 3 * grp + bi
                            t1 = wrk.tile([128, CH], BF16, tag="t1",
                                          name=f"t1_{bi}")
                            nc.vector.tensor_mul(t1, qhats[bi],
                                                 cos_sb[:, s0:s0 + CH])
                            t2 = wrk.tile([128, CH], BF16, tag="t2",
                                          name=f"t2_{bi}")
                            nc.vector.tensor_mul(t2, rots[bi],
                                                 sin_sb[:, s0:s0 + CH])
                            if blk < HPC:
                                qf = qfp.tile([128, CH], BF16,
                                              tag=f"qfin{blk}")
                                qf_state.setdefault(ci, []).append(qf)
                                nc.vector.tensor_add(qf, t1, t2)
                            else:
                                nc.vector.tensor_add(
                                    kv_state[b][0][:, s0:s0 + CH], t1, t2)
                        if grp == 1:
                            # V: transpose [D, S]-chunk to [S, D] tiles
                            vraw = st["raws"][2]
                            v_full = kv_state[b][1]
                            for j in range(CH // 128):
                                tp = paux.tile([128, 128], BF16, tag="s",
                                               name=f"tp{j}")
                                nc.tensor.transpose(
                                    tp, vraw[:, 128 * j:128 * (j + 1)],
                                    iden)
                                nc.vector.tensor_copy(
                                    v_full[:, (CH // 128) * sc + j, :], tp)

                    return {8: stage1, 18: stage2, 26: stage3}

                def emit_epi(ci, grp):
                    for fn in epi_stages(ci, grp).values():
                        fn()

                def emit_attention(ci):
                    """flattened (head, key-tile) stream, 2-deep score
                    lookahead so exp latency hides behind PE work; per-head
                    normalization emitted ~8 steps behind so the DVE
                    reciprocal chain hides behind PE work."""
                    b, sc = chunks[ci]
                    s0 = CH * sc
                    qfin = qf_state.pop(ci)
                    k_full, v_full = kv_state[b]
                    kis = list(range(max(0, 4 * sc - 8), 4 * sc + 4))
                    # masked tiles (longer DVE/ACT chains) first
                    kis.sort(key=lambda ki: 0 if (
                        CH * sc - 128 * ki <= 0 or
                        CH * sc - 128 * ki >= 640) else 1)
                    sum_tiles = {}
                    rcps = {}
                    o_tiles = {}
                    a_uns = {}

                    tasks = [(h, i, ki) for h in range(HPC)
                             for i, ki in enumerate(kis)]
                    pend = []

                    def emit_s(h, i, ki):
                        delta = CH * sc - 128 * ki
                        rel_lo = max(0, -delta)
                        rel_hi = min(CH, 1152 - delta) if delta > 0 else CH
                        s_ps = pss.tile([128, CH], F32, tag="s")
                        nc.tensor.matmul(
                            s_ps[:, rel_lo:rel_hi],
                            k_full[:, 128 * ki:128 * (ki + 1)],
                            qfin[h][:, rel_lo:rel_hi],
                            start=True, stop=True)
                        if delta <= 0:
                            nc.vector.tensor_add(
                                s_ps[:, rel_lo:rel_lo + 128],
                                s_ps[:, rel_lo:rel_lo + 128], mc)
                        elif delta >= 640:
                            nc.vector.tensor_add(
                                s_ps[:, rel_hi - 128:rel_hi],
                                s_ps[:, rel_hi - 128:rel_hi], mw)
                        pr = prp.tile([128, CH], BF16, tag="pr")
                        nc.scalar.activation(
                            pr[:, rel_lo:rel_hi],
                            s_ps[:, rel_lo:rel_hi], EXP)
                        return (h, i, ki, pr, rel_lo, rel_hi)

                    def emit_so(h, i, ki, pr, rel_lo, rel_hi):
                        first, last = (i == 0), (i == len(kis) - 1)
                        if first:
                            o_tiles[h] = pso.tile([128, CH], F32, tag="o",
                                                  name=f"o{h}")
                            sum_tiles[h] = psum_p.tile(
                                [1, CH], F32, tag="sum", name=f"sum{h}")
                        nc.tensor.matmul(
                            sum_tiles[h][:, rel_lo:rel_hi],
                            on128, pr[:, rel_lo:rel_hi],
                            start=first, stop=last)
                        nc.tensor.matmul(
                            o_tiles[h][:, rel_lo:rel_hi],
                            v_full[:, ki, :], pr[:, rel_lo:rel_hi],
                            start=first, stop=last)
                        if last:
                            a_un = aup.tile([128, CH], BF16, tag=f"aun{h}",
                                            name=f"aun{h}")
                            nc.vector.tensor_copy(a_un, o_tiles[h])
                            a_uns[h] = a_un
                            rcp = aup.tile([1, CH], F32, tag="rcp",
                                           name=f"rcp{h}")
                            nc.vector.reciprocal_approx_fast(
                                rcp, sum_tiles[h])
                            rcp_r = aup.tile([1, CH], F32R, tag="rcpr",
                                             name=f"rcpr{h}")
                            nc.vector.tensor_copy(rcp_r, rcp)
                            rcps[h] = rcp_r

                    def make_fin(h):
                        def fin():
                            bc = paux.tile([128, CH], F32, tag="s",
                                           name=f"bc{h}")
                            nc.tensor.matmul(bc, oncol, rcps[h],
                                             start=True, stop=True)
                            nc.vector.tensor_mul(
                                afin[b][:, h, s0:s0 + CH], bc, a_uns[h])
                            if DEBUG and b == 0:
                                nc.sync.dma_start(
                                    dbg_a.ap()[h, :, s0:s0 + CH],
                                    afin[b][:, h, s0:s0 + CH])
                        return fin

                    fin_q = []
                    with nc.named_scope("attn"):
                        for ti, t in enumerate(tasks):
                            pend.append(emit_s(*t))
                            if len(pend) > 3:
                                hh, ii, *_ = pend[0]
                                emit_so(*pend.pop(0))
                                if ii == len(kis) - 1:
                                    fin_q.append((hh, ti))
                            while fin_q and ti - fin_q[0][1] >= 8:
                                deferred.append(make_fin(fin_q.pop(0)[0]))
                                deferred.pop()()
                        for e in pend:
                            hh, ii, *_ = e
                            emit_so(*e)
                            if ii == len(kis) - 1:
                                fin_q.append((hh, 0))
                    # leftover finalizes run inside the next QKV group's
                    # matmul stream (or immediately on the last chunk)
                    for hh, _ in fin_q:
                        deferred.append(make_fin(hh))

                # ---- software-pipelined emission schedule --------------
                load_hs(0)
                load_weights(0)
                load_consts()
                load_weights(1)
                emit_qkv(0, 0)
                emit_qkv(0, 1, mid=epi_stages(0, 0))
                for ci in range(NCH):
                    if ci + 1 < NCH:
                        emit_qkv(ci + 1, 0, mid=epi_stages(ci, 1))
                    else:
                        emit_epi(ci, 1)
                    emit_attention(ci)
                    if ci + 1 < NCH:
                        emit_qkv(ci + 1, 1, mid=epi_stages(ci + 1, 0))
                    if DEBUG and chunks[ci] == (0, NSC - 1):
                        nc.sync.dma_start(dbg_k.ap(), kv_state[0][0])
                        nc.sync.dma_start(dbg_v.ap(), kv_state[0][1])
                while deferred:
                    deferred.pop(0)()

            # ---------------- phase C: output projection -----------------
            with tc.tile_pool(name="wo", bufs=1) as wop, \
                 tc.tile_pool(name="ostg", bufs=4) as ost, \
                 tc.tile_pool(name="ps_c", bufs=8, space="PSUM") as pc:
                wo_sb = wop.tile([128, QW // 128, H], BF16)
                for _cb in range(8):
                    nc.sync.dma_start(
                        wo_sb[:, :, 512 * _cb:512 * (_cb + 1)],
                        wo_s.ap()[:, 512 * _cb:512 * (_cb + 1)]
                            .rearrange("(o p) c -> p o c", p=128))
                NR = QW // 128
                for b in range(B):
                    for st in range(S // 128):
                        # 4 output blocks per group, ring of 8 banks so the
                        # next group's matmuls overlap this group's copies.
                        for g in range(2):
                            hcbs = range(4 * g, 4 * g + 4)
                            c_tiles = {hcb: pc.tile([128, 512], F32, tag="c",
                                                    name=f"c{hcb % 4}")
                                       for hcb in hcbs}
                            for r in range(NR):
                                a_t = afin[b][:, r,
                                              128 * st:128 * (st + 1)]
                                for hcb in hcbs:
                                    nc.tensor.matmul(
                                        c_tiles[hcb], a_t,
                                        wo_sb[:, r,
                                              512 * hcb:512 * (hcb + 1)],
                                        start=(r == 0), stop=(r == NR - 1))
                            for hcb in hcbs:
                                o_sb = ost.tile([128, 512], BF16, tag="ostg")
                                nc.scalar.copy(o_sb, c_tiles[hcb])
                                nc.sync.dma_start(
                                    out_part.ap()[b, 128 * st:128 * (st + 1),
                                                  512 * hcb:512 * (hcb + 1)],
                                    o_sb)

    nc.compile()
    return nc


def _host_prep(hidden_states, wq, wk, wv, wo, q_norm_w, k_norm_w):
    """Build the per-core input maps."""
    import ml_dtypes
    f32 = np.float32
    bf16 = ml_dtypes.bfloat16
    hsT = np.ascontiguousarray(
        np.transpose(hidden_states.astype(f32), (0, 2, 1))).astype(bf16)

    pos = np.arange(S, dtype=np.float64)
    inv_freq = 1.0 / (THETA ** (np.arange(0, D, 2, dtype=np.float64) / D))
    freqs = pos[:, None] * inv_freq[None, :]
    emb = np.concatenate([freqs, freqs], axis=-1)           # [S, D]
    cosT = np.ascontiguousarray(np.cos(emb).T.astype(f32))  # [D, S]
    sinT = np.ascontiguousarray(np.sin(emb).T.astype(f32))

    protT = np.zeros((128, 128), f32)
    protT[64 + np.arange(64), np.arange(64)] = -1.0
    protT[np.arange(64), 64 + np.arange(64)] = 1.0

    kd = np.arange(128)[:, None]
    qd = np.arange(128)[None, :]
    mask_c = np.where(qd >= kd, 0.0, NEG).astype(f32)
    mask_w = np.where(qd < kd, 0.0, NEG).astype(f32)

    common = {
        "hsT": hsT,
        "cosT": cosT.astype(bf16),
        "sinT": sinT.astype(bf16),
        "ones128": np.ones((128, 1), bf16),
        "ones_col": np.ones((1, 128), f32),
        "protT": protT.astype(bf16),
        "idn": np.eye(128, dtype=bf16),
        "mask_c": mask_c,
        "mask_w": mask_w,
        "wrow_q": (q_norm_w.astype(f32) / np.sqrt(D)).reshape(1, D),
        "wrow_k": k_norm_w.astype(f32).reshape(1, D),
    }
    in_maps = []
    for c in range(NCORES):
        m = dict(common)
        m["wq_s"] = np.ascontiguousarray(
            wq[:, QW * c:QW * (c + 1)]).astype(bf16)
        m["wk_s"] = np.ascontiguousarray(
            wk[:, D * c:D * (c + 1)]).astype(bf16)
        m["wv_s"] = np.ascontiguousarray(
            wv[:, D * c:D * (c + 1)]).astype(bf16)
        m["wo_s"] = np.ascontiguousarray(
            wo[QW * c:QW * (c + 1), :]).astype(bf16)
        in_maps.append(m)
    return in_maps


def kernel(hidden_states, wq, wk, wv, wo, q_norm_w, k_norm_w,
           _trace=False, _return_results=False):
    from concourse import bass_utils

    hidden_states = np.asarray(hidden_states)
    wq, wk, wv, wo = (np.asarray(a) for a in (wq, wk, wv, wo))
    q_norm_w, k_norm_w = np.asarray(q_norm_w), np.asarray(k_norm_w)

    if "nc" not in _CACHE:
        _CACHE["nc"] = _build()
    nc = _CACHE["nc"]

    in_maps = _host_prep(hidden_states, wq, wk, wv, wo, q_norm_w, k_norm_w)
    res = bass_utils.run_bass_kernel_spmd(
        nc, in_maps, core_ids=list(range(NCORES)), trace=_trace)

    out = np.zeros((B, S, H), np.float32)
    for c in range(NCORES):
        out += res.results[c]["out_part"].astype(np.float32)
    if _return_results:
        return out, res
    return out


# revision 12
# speedup vs baseline: 1.0173x; 1.0173x over previous
"""Exaone4 attention kernel for 8 Trainium2 NeuronCores.

Sharding: tensor-parallel over heads (TP=8). Core i owns query heads
4i..4i+3 and kv head i (one GQA group), processes both batch elements,
and computes a row-parallel partial of the output projection; the host
sums the 8 partials.

v3: bf16 matmul pipeline (fp32 PSUM accumulation); hidden-state chunk
resident in SBUF; QKV epilogue software-pipelined behind the next
group's matmuls; attention score/sum/PV matmuls trimmed to the exact
sliding-window band (per-element PSUM accumulate); per-head softmax
denominators in 32-strided rows of one PSUM bank, inverted once per
chunk with reciprocal_approx_fast; SBUF-resident attention output;
bf16 partial-output write.

Shapes (hardcoded): B=2, S=2048, H=4096, NH=32, NKV=8, D=128,
WINDOW=1024, eps=1e-5, theta=10000.
"""

import os
import sys

for _p in ("/opt/trn_rl_repo",):
    if _p not in sys.path and os.path.isdir(_p):
        sys.path.insert(0, _p)

import numpy as np

B, S, H = 2, 2048, 4096
NH, NKV, D = 32, 8, 128
WINDOW = 1024
EPS = 1e-5
THETA = 10000.0

NCORES = 8
HPC = NH // NCORES          # query heads per core = 4
QW = HPC * D                # q-proj cols per core = 512
CH = 512                    # sequence chunk
NSC = S // CH               # 4 chunks
HC = H // 128               # 32 contraction chunks
NEG = -1.0e30

_CACHE = {}


def _build():
    import concourse.bass as bass
    import concourse.tile as tile
    from concourse import mybir, bacc

    F32 = mybir.dt.float32
    F32R = mybir.dt.float32r
    BF16 = mybir.dt.bfloat16
    EXP = mybir.ActivationFunctionType.Exp
    RSQRT = mybir.ActivationFunctionType.Abs_reciprocal_sqrt

    nc = bacc.Bacc("TRN2", target_bir_lowering=False, debug=False)

    hsT = nc.dram_tensor("hsT", [B, H, S], BF16, kind="ExternalInput")
    wq_s = nc.dram_tensor("wq_s", [H, QW], BF16, kind="ExternalInput")
    wk_s = nc.dram_tensor("wk_s", [H, D], BF16, kind="ExternalInput")
    wv_s = nc.dram_tensor("wv_s", [H, D], BF16, kind="ExternalInput")
    wo_s = nc.dram_tensor("wo_s", [QW, H], BF16, kind="ExternalInput")
    cosT = nc.dram_tensor("cosT", [D, S], BF16, kind="ExternalInput")
    sinT = nc.dram_tensor("sinT", [D, S], BF16, kind="ExternalInput")
    wrow_q = nc.dram_tensor("wrow_q", [1, D], F32R, kind="ExternalInput")
    wrow_k = nc.dram_tensor("wrow_k", [1, D], F32R, kind="ExternalInput")
    ones128 = nc.dram_tensor("ones128", [128, 1], BF16, kind="ExternalInput")
    ones_col = nc.dram_tensor("ones_col", [1, 128], F32R, kind="ExternalInput")
    protT = nc.dram_tensor("protT", [128, 128], BF16, kind="ExternalInput")
    idn = nc.dram_tensor("idn", [128, 128], BF16, kind="ExternalInput")
    mask_c = nc.dram_tensor("mask_c", [128, 128], F32, kind="ExternalInput")
    mask_w = nc.dram_tensor("mask_w", [128, 128], F32, kind="ExternalInput")
    out_part = nc.dram_tensor("out_part", [B, S, H], BF16,
                              kind="ExternalOutput")

    DEBUG = bool(os.environ.get("BASS_KERNEL_DEBUG"))
    if DEBUG:
        dbg_k = nc.dram_tensor("dbg_k", [128, S], BF16, kind="ExternalOutput")
        dbg_v = nc.dram_tensor("dbg_v", [128, S // 128, 128], BF16,
                               kind="ExternalOutput")
        dbg_q = nc.dram_tensor("dbg_q", [HPC, 128, S], BF16,
                               kind="ExternalOutput")
        dbg_a = nc.dram_tensor("dbg_a", [HPC, 128, S], BF16,
                               kind="ExternalOutput")

    with tile.TileContext(nc) as tc, \
         nc.allow_low_precision(reason="deliberate bf16 matmul pipeline"):
        with tc.tile_pool(name="consts", bufs=1) as consts, \
             tc.tile_pool(name="afin", bufs=1) as afp:
            cos_sb = consts.tile([D, S], BF16)
            nc.sync.dma_start(cos_sb, cosT.ap())
            sin_sb = consts.tile([D, S], BF16)
            nc.sync.dma_start(sin_sb, sinT.ap())
            wq_row = consts.tile([1, D], F32R)
            nc.sync.dma_start(wq_row, wrow_q.ap())
            wk_row = consts.tile([1, D], F32R)
            nc.sync.dma_start(wk_row, wrow_k.ap())
            on128 = consts.tile([128, 1], BF16)
            nc.sync.dma_start(on128, ones128.ap())
            oncol = consts.tile([1, 128], F32R)
            nc.sync.dma_start(oncol, ones_col.ap())
            prot = consts.tile([128, 128], BF16)
            nc.sync.dma_start(prot, protT.ap())
            iden = consts.tile([128, 128], BF16)
            nc.sync.dma_start(iden, idn.ap())
            mc = consts.tile([128, 128], F32)
            nc.sync.dma_start(mc, mask_c.ap())
            mw = consts.tile([128, 128], F32)
            nc.sync.dma_start(mw, mask_w.ap())
            eps_t = consts.tile([1, 1], F32)
            nc.vector.memset(eps_t, EPS)

            # attention output, SBUF-resident across phases A+B and C
            afin = [afp.tile([128, HPC, S], BF16, tag=f"af{b}",
                             name=f"af{b}") for b in range(B)]

            # ---------------- phases A+B: QKV + norm/rope + attention ----
            with tc.tile_pool(name="wqkv", bufs=1) as wp, \
                 tc.tile_pool(name="kv", bufs=1) as kvp, \
                 tc.tile_pool(name="work", bufs=2) as wrk, \
                 tc.tile_pool(name="epi", bufs=1) as epi, \
                 tc.tile_pool(name="hs", bufs=2) as hsp, \
                 tc.tile_pool(name="probs", bufs=3) as prp, \
                 tc.tile_pool(name="qf", bufs=2) as qfp, \
                 tc.tile_pool(name="aun", bufs=1) as aup, \
                 tc.tile_pool(name="ps_qkv", bufs=1, space="PSUM") as pq, \
                 tc.tile_pool(name="ps_s", bufs=3, space="PSUM") as pss, \
                 tc.tile_pool(name="ps_o", bufs=1, space="PSUM") as pso, \
                 tc.tile_pool(name="ps_sum", bufs=1, space="PSUM") as psum_p:
                paux = pss

                wq_sb = wp.tile([128, HC, QW], BF16)
                for _blk in range(HPC):
                    nc.sync.dma_start(
                        wq_sb[:, :, 128 * _blk:128 * (_blk + 1)],
                        wq_s.ap()[:, 128 * _blk:128 * (_blk + 1)]
                            .rearrange("(o p) c -> p o c", p=128))
                wk_sb = wp.tile([128, HC, D], BF16)
                nc.sync.dma_start(
                    wk_sb, wk_s.ap().rearrange("(o p) c -> p o c", p=128))
                wv_sb = wp.tile([128, HC, D], BF16)
                nc.sync.dma_start(
                    wv_sb, wv_s.ap().rearrange("(o p) c -> p o c", p=128))

                chunks = [(b, sc) for b in range(B) for sc in range(NSC)]
                NCH = len(chunks)
                hs_state, raw_state, qf_state = {}, {}, {}
                kv_state = {}
                deferred = []

                def load_hs(ci):
                    b, sc = chunks[ci]
                    t = hsp.tile([128, HC, CH], BF16, tag="hst")
                    nc.sync.dma_start(
                        t, hsT.ap()[b, :, CH * sc:CH * (sc + 1)]
                            .rearrange("(o p) c -> p o c", p=128))
                    hs_state[ci] = t

                def emit_qkv(ci, grp, mid=None):
                    """QKV matmuls for 3 col-blocks + PSUM drain copies."""
                    b, sc = chunks[ci]
                    if grp == 0 and ci + 1 < NCH:
                        load_hs(ci + 1)
                    hs_sb = hs_state[ci] if grp == 0 else hs_state.pop(ci)
                    with nc.named_scope("qkv"):
                        qkv_ps = pq.tile([128, 3, CH], F32, tag="qkv",
                                         name=f"qkv{ci}_{grp}")
                        for hc in range(HC):
                            if hc == 4:
                                while deferred:
                                    deferred.pop(0)()
                            if hc == 12 and mid is not None:
                                mid()
                            for bi in range(3):
                                blk = 3 * grp + bi
                                if blk < HPC:
                                    lhs = wq_sb[:, hc,
                                                128 * blk:128 * (blk + 1)]
                                elif blk == HPC:
                                    lhs = wk_sb[:, hc, :]
                                else:
                                    lhs = wv_sb[:, hc, :]
                                nc.tensor.matmul(
                                    qkv_ps[:, bi, :], lhs, hs_sb[:, hc, :],
                                    start=(hc == 0), stop=(hc == HC - 1))
                        raws = {}
                        for bi in range(3):
                            raw = epi.tile([128, CH], BF16,
                                           tag=f"raw{grp}{bi}",
                                           name=f"raw{grp}{bi}")
                            nc.vector.tensor_copy(raw, qkv_ps[:, bi, :])
                            raws[bi] = raw
                        raw_state[(ci, grp)] = raws

                def epi_stages(ci, grp):
                    """norm + rope for one group's blocks, split into 3
                    stages so the PE ops spread through the host matmul
                    stream (ACT rsqrt latency hides between stages)."""
                    b, sc = chunks[ci]
                    s0 = CH * sc
                    st = {}

                    def stage1():
                        raws = raw_state.pop((ci, grp))
                        st["raws"] = raws
                        if grp == 1 and sc == 0:
                            k_full = kvp.tile([128, S], BF16, tag="k_full",
                                              name=f"k{b}")
                            v_full = kvp.tile([128, S // 128, 128], BF16,
                                              tag="v_full", name=f"v{b}")
                            kv_state[b] = (k_full, v_full)
                        norm_bis = list(range(3)) if grp == 0 else [0, 1]
                        st["norm_bis"] = norm_bis
                        sqs, ssqs, rinvs = {}, {}, {}
                        for bi in norm_bis:
                            sq = epi.tile([128, CH], BF16, tag=f"sq{bi}",
                                          name=f"sq{bi}")
                            nc.vector.tensor_mul(sq, raws[bi], raws[bi])
                            sqs[bi] = sq
                        for bi in norm_bis:
                            ssq = paux.tile([1, CH], F32, tag="s",
                                            name=f"ssq{bi}")
                            nc.tensor.matmul(ssq, on128, sqs[bi],
                                             start=True, stop=True)
                            ssqs[bi] = ssq
                        for bi in norm_bis:
                            rinv = epi.tile([1, CH], F32R, tag=f"rinv{bi}",
                                            name=f"rinv{bi}")
                            nc.scalar.activation(rinv, ssqs[bi], RSQRT,
                                                 bias=eps_t, scale=1.0 / D)
                            rinvs[bi] = rinv
                        st["rinvs"] = rinvs

                    def stage2():
                        raws, rinvs = st["raws"], st["rinvs"]
                        wtils, qhats = {}, {}
                        for bi in st["norm_bis"]:
                            wrow = (wq_row if 3 * grp + bi < HPC
                                    else wk_row)
                            wtil = paux.tile([128, CH], F32, tag="s",
                                             name=f"wtil{bi}")
                            nc.tensor.matmul(wtil, wrow, rinvs[bi],
                                             start=True, stop=True)
                            wtils[bi] = wtil
                        for bi in st["norm_bis"]:
                            qhat = epi.tile([128, CH], BF16,
                                            tag=f"qhat{bi}",
                                            name=f"qhat{bi}")
                            nc.vector.tensor_mul(qhat, wtils[bi], raws[bi])
                            qhats[bi] = qhat
                        st["qhats"] = qhats

                    def stage3():
                        raws, qhats = st["raws"], st["qhats"]
                        rots = {}
                        for bi in st["norm_bis"]:
                            rot = paux.tile([128, CH], F32, tag="s",
                                            name=f"rot{bi}")
                            nc.tensor.matmul(rot, prot, qhats[bi],
                                             start=True, stop=True)
                            rots[bi] = rot
                        for bi in st["norm_bis"]:
                            blk = 3 * grp + bi
                            t1 = wrk.tile([128, CH], BF16, tag="t1",
                                          name=f"t1_{bi}")
                            nc.vector.tensor_mul(t1, qhats[bi],
                                                 cos_sb[:, s0:s0 + CH])
                            t2 = wrk.tile([128, CH], BF16, tag="t2",
                                          name=f"t2_{bi}")
                            nc.vector.tensor_mul(t2, rots[bi],
                                                 sin_sb[:, s0:s0 + CH])
                            if blk < HPC:
                                qf = qfp.tile([128, CH], BF16,
                                              tag=f"qfin{blk}")
                                qf_state.setdefault(ci, []).append(qf)
                                nc.vector.tensor_add(qf, t1, t2)
                            else:
                                nc.vector.tensor_add(
                                    kv_state[b][0][:, s0:s0 + CH], t1, t2)
                        if grp == 1:
                            # V: transpose [D, S]-chunk to [S, D] tiles
                            vraw = st["raws"][2]
                            v_full = kv_state[b][1]
                            for j in range(CH // 128):
                                tp = paux.tile([128, 128], BF16, tag="s",
                                               name=f"tp{j}")
                                nc.tensor.transpose(
                                    tp, vraw[:, 128 * j:128 * (j + 1)],
                                    iden)
                                nc.vector.tensor_copy(
                                    v_full[:, (CH // 128) * sc + j, :], tp)

                    return {8: stage1, 18: stage2, 26: stage3}

                def emit_epi(ci, grp):
                    for fn in epi_stages(ci, grp).values():
                        fn()

                def emit_attention(ci):
                    """flattened (head, key-tile) stream, 2-deep score
                    lookahead so exp latency hides behind PE work; per-head
                    normalization emitted ~8 steps behind so the DVE
                    reciprocal chain hides behind PE work."""
                    b, sc = chunks[ci]
                    s0 = CH * sc
                    qfin = qf_state.pop(ci)
                    k_full, v_full = kv_state[b]
                    kis = list(range(max(0, 4 * sc - 8), 4 * sc + 4))
                    # masked tiles (longer DVE/ACT chains) first
                    kis.sort(key=lambda ki: 0 if (
                        CH * sc - 128 * ki <= 0 or
                        CH * sc - 128 * ki >= 640) else 1)
                    sum_tiles = {}
                    rcps = {}
                    o_tiles = {}
                    a_uns = {}

                    tasks = [(h, i, ki) for h in range(HPC)
                             for i, ki in enumerate(kis)]
                    pend = []

                    def emit_s(h, i, ki):
                        delta = CH * sc - 128 * ki
                        rel_lo = max(0, -delta)
                        rel_hi = min(CH, 1152 - delta) if delta > 0 else CH
                        s_ps = pss.tile([128, CH], F32, tag="s")
                        nc.tensor.matmul(
                            s_ps[:, rel_lo:rel_hi],
                            k_full[:, 128 * ki:128 * (ki + 1)],
                            qfin[h][:, rel_lo:rel_hi],
                            start=True, stop=True)
                        if delta <= 0:
                            nc.vector.tensor_add(
                                s_ps[:, rel_lo:rel_lo + 128],
                                s_ps[:, rel_lo:rel_lo + 128], mc)
                        elif delta >= 640:
                            nc.vector.tensor_add(
                                s_ps[:, rel_hi - 128:rel_hi],
                                s_ps[:, rel_hi - 128:rel_hi], mw)
                        pr = prp.tile([128, CH], BF16, tag="pr")
                        nc.scalar.activation(
                            pr[:, rel_lo:rel_hi],
                            s_ps[:, rel_lo:rel_hi], EXP)
                        return (h, i, ki, pr, rel_lo, rel_hi)

                    def emit_so(h, i, ki, pr, rel_lo, rel_hi):
                        first, last = (i == 0), (i == len(kis) - 1)
                        if first:
                            o_tiles[h] = pso.tile([128, CH], F32, tag="o",
                                                  name=f"o{h}")
                            sum_tiles[h] = psum_p.tile(
                                [1, CH], F32, tag="sum", name=f"sum{h}")
                        nc.tensor.matmul(
                            sum_tiles[h][:, rel_lo:rel_hi],
                            on128, pr[:, rel_lo:rel_hi],
                            start=first, stop=last)
                        nc.tensor.matmul(
                            o_tiles[h][:, rel_lo:rel_hi],
                            v_full[:, ki, :], pr[:, rel_lo:rel_hi],
                            start=first, stop=last)
                        if last:
                            a_un = aup.tile([128, CH], BF16, tag=f"aun{h}",
                                            name=f"aun{h}")
                            nc.vector.tensor_copy(a_un, o_tiles[h])
                            a_uns[h] = a_un
                            rcp = aup.tile([1, CH], F32, tag="rcp",
                                           name=f"rcp{h}")
                            nc.vector.reciprocal_approx_fast(
                                rcp, sum_tiles[h])
                            rcp_r = aup.tile([1, CH], F32R, tag="rcpr",
                                             name=f"rcpr{h}")
                            nc.vector.tensor_copy(rcp_r, rcp)
                            rcps[h] = rcp_r

                    def make_fin(h):
                        def fin():
                            bc = paux.tile([128, CH], F32, tag="s",
                                           name=f"bc{h}")
                            nc.tensor.matmul(bc, oncol, rcps[h],
                                             start=True, stop=True)
                            nc.vector.tensor_mul(
                                afin[b][:, h, s0:s0 + CH], bc, a_uns[h])
                            if DEBUG and b == 0:
                                nc.sync.dma_start(
                                    dbg_a.ap()[h, :, s0:s0 + CH],
                                    afin[b][:, h, s0:s0 + CH])
                        return fin

                    fin_q = []
                    with nc.named_scope("attn"):
                        for ti, t in enumerate(tasks):
                            pend.append(emit_s(*t))
                            if len(pend) > 3:
                                hh, ii, *_ = pend[0]
                                emit_so(*pend.pop(0))
                                if ii == len(kis) - 1:
                                    fin_q.append((hh, ti))
                            while fin_q and ti - fin_q[0][1] >= 8:
                                deferred.append(make_fin(fin_q.pop(0)[0]))
                                deferred.pop()()
                        for e in pend:
                            hh, ii, *_ = e
                            emit_so(*e)
                            if ii == len(kis) - 1:
                                fin_q.append((hh, 0))
                    # leftover finalizes run inside the next QKV group's
                    # matmul stream (or immediately on the last chunk)
                    for hh, _ in fin_q:
                        deferred.append(make_fin(hh))

                # ---- software-pipelined emission schedule --------------
                load_hs(0)
                emit_qkv(0, 0)
                emit_qkv(0, 1)
                emit_epi(0, 0)
                for ci in range(NCH):
                    if ci + 1 < NCH:
                        emit_qkv(ci + 1, 0, mid=lambda: emit_epi(ci, 1))
                    else:
                        emit_epi(ci, 1)
                    emit_attention(ci)
                    if ci + 1 < NCH:
                        emit_qkv(ci + 1, 1)
                        emit_epi(ci + 1, 0)
                    if DEBUG and chunks[ci] == (0, NSC - 1):
                        nc.sync.dma_start(dbg_k.ap(), kv_state[0][0])
                        nc.sync.dma_start(dbg_v.ap(), kv_state[0][1])
                while deferred:
                    deferred.pop(0)()

            # ---------------- phase C: output projection -----------------
            with tc.tile_pool(name="wo", bufs=1) as wop, \
                 tc.tile_pool(name="ostg", bufs=4) as ost, \
                 tc.tile_pool(name="ps_c", bufs=8, space="PSUM") as pc:
                wo_sb = wop.tile([128, QW // 128, H], BF16)
                for _cb in range(8):
                    nc.sync.dma_start(
                        wo_sb[:, :, 512 * _cb:512 * (_cb + 1)],
                        wo_s.ap()[:, 512 * _cb:512 * (_cb + 1)]
                            .rearrange("(o p) c -> p o c", p=128))
                NR = QW // 128
                for b in range(B):
                    for st in range(S // 128):
                        # 4 output blocks per group, ring of 8 banks so the
                        # next group's matmuls overlap this group's copies.
                        for g in range(2):
                            hcbs = range(4 * g, 4 * g + 4)
                            c_tiles = {hcb: pc.tile([128, 512], F32, tag="c",
                                                    name=f"c{hcb % 4}")
                                       for hcb in hcbs}
                            for r in range(NR):
                                a_t = afin[b][:, r,
                                              128 * st:128 * (st + 1)]
                                for hcb in hcbs:
                                    nc.tensor.matmul(
                                        c_tiles[hcb], a_t,
                                        wo_sb[:, r,
                                              512 * hcb:512 * (hcb + 1)],
                                        start=(r == 0), stop=(r == NR - 1))
                            for hcb in hcbs:
                                o_sb = ost.tile([128, 512], BF16, tag="ostg")
                                nc.scalar.copy(o_sb, c_tiles[hcb])
                                nc.sync.dma_start(
                                    out_part.ap()[b, 128 * st:128 * (st + 1),
                                                  512 * hcb:512 * (hcb + 1)],
                                    o_sb)

    nc.compile()
    return nc


def _host_prep(hidden_states, wq, wk, wv, wo, q_norm_w, k_norm_w):
    """Build the per-core input maps."""
    import ml_dtypes
    f32 = np.float32
    bf16 = ml_dtypes.bfloat16
    hsT = np.ascontiguousarray(
        np.transpose(hidden_states.astype(f32), (0, 2, 1))).astype(bf16)

    pos = np.arange(S, dtype=np.float64)
    inv_freq = 1.0 / (THETA ** (np.arange(0, D, 2, dtype=np.float64) / D))
    freqs = pos[:, None] * inv_freq[None, :]
    emb = np.concatenate([freqs, freqs], axis=-1)           # [S, D]
    cosT = np.ascontiguousarray(np.cos(emb).T.astype(f32))  # [D, S]
    sinT = np.ascontiguousarray(np.sin(emb).T.astype(f32))

    protT = np.zeros((128, 128), f32)
    protT[64 + np.arange(64), np.arange(64)] = -1.0
    protT[np.arange(64), 64 + np.arange(64)] = 1.0

    kd = np.arange(128)[:, None]
    qd = np.arange(128)[None, :]
    mask_c = np.where(qd >= kd, 0.0, NEG).astype(f32)
    mask_w = np.where(qd < kd, 0.0, NEG).astype(f32)

    common = {
        "hsT": hsT,
        "cosT": cosT.astype(bf16),
        "sinT": sinT.astype(bf16),
        "ones128": np.ones((128, 1), bf16),
        "ones_col": np.ones((1, 128), f32),
        "protT": protT.astype(bf16),
        "idn": np.eye(128, dtype=bf16),
        "mask_c": mask_c,
        "mask_w": mask_w,
        "wrow_q": (q_norm_w.astype(f32) / np.sqrt(D)).reshape(1, D),
        "wrow_k": k_norm_w.astype(f32).reshape(1, D),
    }
    in_maps = []
    for c in range(NCORES):
        m = dict(common)
        m["wq_s"] = np.ascontiguousarray(
            wq[:, QW * c:QW * (c + 1)]).astype(bf16)
        m["wk_s"] = np.ascontiguousarray(
            wk[:, D * c:D * (c + 1)]).astype(bf16)
        m["wv_s"] = np.ascontiguousarray(
            wv[:, D * c:D * (c + 1)]).astype(bf16)
        m["wo_s"] = np.ascontiguousarray(
            wo[QW * c:QW * (c + 1), :]).astype(bf16)
        in_maps.append(m)
    return in_maps


def kernel(hidden_states, wq, wk, wv, wo, q_norm_w, k_norm_w,
           _trace=False, _return_results=False):
    from concourse import bass_utils

    hidden_states = np.asarray(hidden_states)
    wq, wk, wv, wo = (np.asarray(a) for a in (wq, wk, wv, wo))
    q_norm_w, k_norm_w = np.asarray(q_norm_w), np.asarray(k_norm_w)

    if "nc" not in _CACHE:
        _CACHE["nc"] = _build()
    nc = _CACHE["nc"]

    in_maps = _host_prep(hidden_states, wq, wk, wv, wo, q_norm_w, k_norm_w)
    res = bass_utils.run_bass_kernel_spmd(
        nc, in_maps, core_ids=list(range(NCORES)), trace=_trace)

    out = np.zeros((B, S, H), np.float32)
    for c in range(NCORES):
        out += res.results[c]["out_part"].astype(np.float32)
    if _return_results:
        return out, res
    return out
